# revision 69
# baseline (speedup 1.0000x reference)
"""CenterLoss kernel for 8 Trainium2 NeuronCores.

reference:
    w_t = weight[targets]                    # [N, D] gather
    d   = sqrt(sum((x - w_t)^2, axis=1) + 1e-6)
    out = mean(d)

Strategy (data-parallel over N):
  - Shard x/targets along N across 8 cores (8192 rows each); the small
    class-center table is replicated.
  - The per-row center fetch is split across two engines that run in
    parallel:
      * PE path (first NPE chunks): the fp8-e4m3 table lives in SBUF as
        8 class-chunk tiles [128cls, 512]; for each group of 128 rows a
        host-built one-hot selection matrix [128cls, 128row] (fp8, per
        class chunk) is matmul-accumulated against the table, producing
        w_t for those rows in PSUM exactly (one-hot select).
      * Q7 path (remaining chunks): dma_gather fetches bf16 rows from a
        host-converted bf16 table in HBM (indices pre-permuted on host
        so gather slot t*128+p == x row p*64+off+t).
    The dtype compressions change the final mean by ~1e-7..1e-6
    relative (measured on the reference distribution).
  - PE chunks use the expansion s = ||x||^2 - 2 x.w + ||w||^2: ACT
    squares x directly (in its otherwise-idle startup window), DVE does
    one fused x*w multiply+row-accumulate against the PSUM, and the
    host adds ||w||^2 (it knows targets).  Gather chunks compute
    diff = x - w on DVE (bf16) and square+row-reduce split between ACT
    and DVE.
  - Host: combine partials, sqrt(s + eps), mean over N (<0.01% of the
    FLOPs).
"""

import numpy as np
import ml_dtypes

import concourse.bacc as bacc
import concourse.bass as bass
import concourse.mybir as mybir
from concourse.bass_utils import run_bass_kernel_spmd
from concourse.tile import TileContext

N, D, C = 65536, 512, 1000
NCORES = 8
NSH = N // NCORES            # 8192 rows per core
P = 128
TPB = NSH // P               # 64 row-groups per partition
CHUNK_T = 8                  # row-groups per chunk
NCHUNK = TPB // CHUNK_T      # 8 chunks
CHUNK_ROWS = P * CHUNK_T     # 1024 rows per chunk
IDX_COLS = NSH // 16         # int16 columns of wrapped indices
NPE = 4                      # chunks resolved on the PE
# chunk schedule: (kind, row-groups).  PE chunks go first (the PE can
# start as soon as its table + first one-hots land, ~15us, while the
# first dma_gather waits ~23us for the Q7 ucode overlay); the last two
# gather chunks are halved to shorten the pipeline drain.
CHUNKS = [("pe", 8)] * 3 + [("g", 8)] * 4 + [("pe", 4)] + [("g", 4)]
assert sum(ct for _, ct in CHUNKS) == TPB
assert sum(1 for k, _ in CHUNKS if k == "pe") == NPE
assert all(ct <= CHUNK_T for k, ct in CHUNKS if k == "pe")
KCH = 8                      # class chunks (1024 padded classes / 128)
CPAD = KCH * P               # padded class count
N_DVE_SQ = 2                 # row-groups per chunk squared on DVE (rest ACT)
EPS = 1e-6

_dt = mybir.dt


def _build_bass() -> bass.Bass:
    nc = bacc.Bacc(trn_type="TRN2")
    x_d = nc.dram_tensor("x", [NSH, D], _dt.float32, kind="ExternalInput")
    wq_d = nc.dram_tensor("wq", [C, D], _dt.bfloat16, kind="ExternalInput")
    # w8 pre-arranged on host into SBUF layout: [p, k*D] with
    # w8[p, k*D:(k+1)*D] = fp8(weight[128k + p])
    w8_d = nc.dram_tensor("w8", [P, KCH * D], _dt.float8e4, kind="ExternalInput")
    oh_d = nc.dram_tensor(
        "oh", [P, NPE * CHUNK_T * KCH * P], _dt.float8e4, kind="ExternalInput"
    )
    idx_d = nc.dram_tensor("idx", [P, IDX_COLS], _dt.int16, kind="ExternalInput")
    out_d = nc.dram_tensor("out", [P, TPB], _dt.float32, kind="ExternalOutput")
    # second output: ||x||^2 per row-group (PE-chunk columns only)
    xx_d = nc.dram_tensor("xx", [P, TPB], _dt.float32, kind="ExternalOutput")

    # partition p <-> rows p*TPB + t for t in [0, TPB)
    x_v = x_d[:, :].rearrange("(p t) d -> p t d", p=P)
    # one-hot input, partition = class-in-chunk: [cls, c, t, k, row]
    oh_v = oh_d[:, :].rearrange(
        "p (c t k r) -> p c t k r", c=NPE, t=CHUNK_T, k=KCH
    )

    with TileContext(nc) as tc:
        with (
            tc.tile_pool(name="xin", bufs=4) as x_pool,
            tc.tile_pool(name="wq", bufs=5) as wq_pool,
            tc.tile_pool(name="scr", bufs=3) as scr_pool,
            tc.tile_pool(name="psum", bufs=8, space="PSUM") as psum_pool,
            tc.tile_pool(name="oh", bufs=3) as oh_pool,
            tc.tile_pool(name="small", bufs=1) as small,
        ):
            # fp8 table resident in SBUF: wtab[p, k, :] = w8[128k + p]
            wtab = small.tile([P, KCH, D], _dt.float8e4)
            nc.scalar.dma_start(
                out=wtab[:],
                in_=w8_d[:, :].rearrange("p (k d) -> p k d", k=KCH),
            )
            idx_t = small.tile([P, IDX_COLS], _dt.int16)
            nc.scalar.dma_start(out=idx_t[:], in_=idx_d[:, :])
            ssq = small.tile([P, TPB], _dt.float32)
            xxt = small.tile([P, TPB], _dt.float32)

            g0 = 0   # running row-group index
            pe_i = 0  # PE chunk ordinal
            for c, (kind, ct) in enumerate(CHUNKS):
                if kind == "pe":
                    # one-hot load issued before the (2 MB) x load so the
                    # PE feed is not FIFO-queued behind it
                    oh_t = oh_pool.tile([P, ct, KCH, P], _dt.float8e4)
                    nc.sync.dma_start(out=oh_t[:], in_=oh_v[:, pe_i, :ct])
                    pe_i += 1
                x_t = x_pool.tile([P, ct, D], _dt.float32, tag="x")
                nc.sync.dma_start(out=x_t[:], in_=x_v[:, g0 : g0 + ct, :])
                # algebra path everywhere: s = ||x||^2 - 2 x.w + ||w||^2.
                # ACT squares x as soon as it lands (needs neither psums
                # nor gather data, so it runs ahead of the fetch paths);
                # DVE does one fused x*w multiply+row-accumulate per
                # row-group (in place on the x slice); the host adds
                # ||w||^2 from targets.
                for t in range(ct):
                    g = g0 + t
                    sq_t = scr_pool.tile([P, D], _dt.bfloat16, tag="sq")
                    nc.scalar.activation(
                        out=sq_t[:],
                        in_=x_t[:, t, :],
                        func=mybir.ActivationFunctionType.Square,
                        accum_out=xxt[:, g : g + 1],
                    )
                if kind == "pe":
                    for t in range(ct):
                        g = g0 + t
                        ps = psum_pool.tile([P, D], _dt.float32, tag="ps")
                        for k2 in range(KCH // 2):
                            nc.tensor.matmul(
                                out=ps[:],
                                lhsT=oh_t[:, t, 2 * k2 : 2 * k2 + 2, :],
                                rhs=wtab[:, 2 * k2 : 2 * k2 + 2, :],
                                start=(k2 == 0),
                                stop=(k2 == KCH // 2 - 1),
                                perf_mode=mybir.MatmulPerfMode.DoubleRow,
                            )
                        nc.vector.scalar_tensor_tensor(
                            out=x_t[:, t, :],
                            in0=x_t[:, t, :],
                            scalar=0.0,
                            in1=ps[:],
                            op0=mybir.AluOpType.bypass,
                            op1=mybir.AluOpType.mult,
                            accum_out=ssq[:, g : g + 1],
                        )
                    g0 += ct
                    continue
                else:
                    icol0 = (g0 * P) // 16
                    icols = ct * P // 16
                    w_t = wq_pool.tile([P, ct, D], _dt.bfloat16, tag="wq")
                    nc.gpsimd.dma_gather(
                        out_ap=w_t[:],
                        in_ap=wq_d[:, :],
                        idxs_ap=idx_t[:, icol0 : icol0 + icols],
                        num_idxs=ct * P,
                        num_idxs_reg=ct * P,
                        elem_size=D,
                    )
                    for t in range(ct):
                        g = g0 + t
                        nc.vector.scalar_tensor_tensor(
                            out=x_t[:, t, :],
                            in0=x_t[:, t, :],
                            scalar=0.0,
                            in1=w_t[:, t, :],
                            op0=mybir.AluOpType.bypass,
                            op1=mybir.AluOpType.mult,
                            accum_out=ssq[:, g : g + 1],
                        )
                g0 += ct

            # ship per-row partials; host does the combine + sqrt + mean
            nc.sync.dma_start(out=out_d[:, :], in_=ssq[:])
            nc.sync.dma_start(out=xx_d[:, :], in_=xxt[:])
    nc.finalize()
    return nc


def _wrap_indices(targets_shard: np.ndarray) -> np.ndarray:
    """dma_gather index tensor [128, NSH//16] int16 (see docstring in
    the gather branch; only the Q7-path chunks' columns are used)."""
    tg = targets_shard.reshape(P, TPB)
    idx = np.zeros((P, IDX_COLS), np.int16)
    g0 = 0
    for kind, ct in CHUNKS:
        if kind == "g":
            arr = tg[:, g0 : g0 + ct].T.reshape(-1)  # slot-ordered
            wrap = arr.reshape(-1, 16).T             # [16, ct*8]
            c0 = (g0 * P) // 16
            idx[:, c0 : c0 + ct * P // 16] = np.tile(wrap, (8, 1))
        g0 += ct
    return idx


def _build_onehots(targets_shard: np.ndarray) -> np.ndarray:
    """One-hot selectors for the PE chunks: oh[cls, pe_i, t, k, p] = 1
    iff targets[row p*TPB + g0(pe_i) + t] == 128k + cls, flattened to
    [128, NPE*CHUNK_T*KCH*128] fp8."""
    tg = targets_shard.reshape(P, TPB)
    pe_spans = []
    g0 = 0
    for kind, ct in CHUNKS:
        if kind == "pe":
            pe_spans.append((g0, ct))
        g0 += ct
    oh = np.zeros((P, NPE, CHUNK_T, KCH, P), np.uint8)
    for pe_i, (s, ct) in enumerate(pe_spans):
        tcl = tg[:, s : s + ct]                  # [p, t]
        k = tcl // P
        cls = tcl % P
        pp, tt = np.meshgrid(np.arange(P), np.arange(ct), indexing="ij")
        oh[cls.ravel(), pe_i, tt.ravel(), k.ravel(), pp.ravel()] = 1
    one = np.uint8(ml_dtypes.float8_e4m3(1.0).view(np.uint8))
    return (oh * one).reshape(P, -1).view(ml_dtypes.float8_e4m3)


_NC_CACHE = None


def kernel(x, weight, targets):
    global _NC_CACHE
    x = np.ascontiguousarray(np.asarray(x, dtype=np.float32))
    weight = np.ascontiguousarray(np.asarray(weight, dtype=np.float32))
    targets = np.asarray(targets).astype(np.int64)
    assert x.shape == (N, D) and weight.shape == (C, D) and targets.shape == (N,)

    if _NC_CACHE is None:
        _NC_CACHE = _build_bass()
    nc = _NC_CACHE

    wq = np.ascontiguousarray(weight.astype(ml_dtypes.bfloat16))
    w8pad = np.zeros((CPAD, D), ml_dtypes.float8_e4m3)
    w8pad[:C] = weight.astype(ml_dtypes.float8_e4m3)
    # [p, k*D] SBUF layout: row p holds classes p, 128+p, ...
    w8 = np.ascontiguousarray(
        w8pad.reshape(KCH, P, D).transpose(1, 0, 2).reshape(P, KCH * D)
    )
    in_maps = []
    for k in range(NCORES):
        sl = slice(k * NSH, (k + 1) * NSH)
        tsh = targets[sl]
        in_maps.append(
            {
                "x": x[sl],
                "wq": wq,
                "w8": w8,
                "oh": _build_onehots(tsh),
                "idx": _wrap_indices(tsh),
            }
        )

    res = run_bass_kernel_spmd(nc, in_maps, core_ids=list(range(NCORES)))
    wsq = (weight.astype(np.float64) ** 2).sum(1)
    total = np.float64(0.0)
    for k, r in enumerate(res.results):
        tg = targets[k * NSH : (k + 1) * NSH].reshape(P, TPB)
        xw = r["out"].astype(np.float64)
        xx = r["xx"].astype(np.float64)
        s = xx - 2.0 * xw + wsq[tg]
        total += np.sqrt(s + EPS).sum()
    return np.float32(total / N)


if __name__ == "__main__":
    rng = np.random.default_rng(0)
    x = rng.standard_normal((N, D), dtype=np.float32)
    w = (rng.standard_normal((C, D)) / np.sqrt(D)).astype(np.float32)
    t = rng.integers(0, C, size=(N,)).astype(np.int64)
    got = kernel(x, w, t)
    wt = w[t]
    exp = np.sqrt(((x - wt) ** 2).sum(1) + EPS).mean()
    print("kernel:", got, "expected:", exp, "rel:", abs(got - exp) / abs(exp))


# revision 70
# speedup vs baseline: 1.0005x; 1.0005x over previous
"""CenterLoss kernel for 8 Trainium2 NeuronCores.

reference:
    w_t = weight[targets]                    # [N, D] gather
    d   = sqrt(sum((x - w_t)^2, axis=1) + 1e-6)
    out = mean(d)

Strategy (data-parallel over N):
  - Shard x/targets along N across 8 cores (8192 rows each); the small
    class-center table is replicated.
  - The per-row center fetch is split across two engines that run in
    parallel:
      * PE path (first NPE chunks): the fp8-e4m3 table lives in SBUF as
        8 class-chunk tiles [128cls, 512]; for each group of 128 rows a
        host-built one-hot selection matrix [128cls, 128row] (fp8, per
        class chunk) is matmul-accumulated against the table, producing
        w_t for those rows in PSUM exactly (one-hot select).
      * Q7 path (remaining chunks): dma_gather fetches bf16 rows from a
        host-converted bf16 table in HBM (indices pre-permuted on host
        so gather slot t*128+p == x row p*64+off+t).
    The dtype compressions change the final mean by ~1e-7..1e-6
    relative (measured on the reference distribution).
  - All chunks use the expansion s = ||x||^2 - 2 x.w + ||w||^2: ACT
    squares x as soon as it lands (independent of both fetch paths),
    DVE does one fused x*w multiply+row-accumulate per row-group
    against the PSUM (PE chunks) or the gathered bf16 rows (Q7
    chunks), and the host adds ||w||^2 (it knows targets).
  - Host: combine partials, sqrt(s + eps), mean over N (<0.01% of the
    FLOPs).
"""

import numpy as np
import ml_dtypes

import concourse.bacc as bacc
import concourse.bass as bass
import concourse.mybir as mybir
from concourse.bass_utils import run_bass_kernel_spmd
from concourse.tile import TileContext

N, D, C = 65536, 512, 1000
NCORES = 8
NSH = N // NCORES            # 8192 rows per core
P = 128
TPB = NSH // P               # 64 row-groups per partition
CHUNK_T = 8                  # row-groups per chunk
NCHUNK = TPB // CHUNK_T      # 8 chunks
CHUNK_ROWS = P * CHUNK_T     # 1024 rows per chunk
IDX_COLS = NSH // 16         # int16 columns of wrapped indices
NPE = 4                      # chunks resolved on the PE
# chunk schedule: (kind, row-groups).  PE chunks go first (the PE can
# start as soon as its table + first one-hots land, ~15us, while the
# first dma_gather waits ~23us for the Q7 ucode overlay); the last two
# gather chunks are halved to shorten the pipeline drain.
CHUNKS = [("pe", 8)] * 3 + [("g", 8)] * 4 + [("pe", 4)] + [("g", 4)]
assert sum(ct for _, ct in CHUNKS) == TPB
assert sum(1 for k, _ in CHUNKS if k == "pe") == NPE
assert all(ct <= CHUNK_T for k, ct in CHUNKS if k == "pe")
KCH = 8                      # class chunks (1024 padded classes / 128)
CPAD = KCH * P               # padded class count
N_DVE_SQ = 2                 # row-groups per chunk squared on DVE (rest ACT)
EPS = 1e-6

_dt = mybir.dt


def _build_bass() -> bass.Bass:
    nc = bacc.Bacc(trn_type="TRN2")
    x_d = nc.dram_tensor("x", [NSH, D], _dt.float32, kind="ExternalInput")
    wq_d = nc.dram_tensor("wq", [C, D], _dt.bfloat16, kind="ExternalInput")
    # w8 pre-arranged on host into SBUF layout: [p, k*D] with
    # w8[p, k*D:(k+1)*D] = fp8(weight[128k + p])
    w8_d = nc.dram_tensor("w8", [P, KCH * D], _dt.float8e4, kind="ExternalInput")
    oh_d = nc.dram_tensor(
        "oh", [P, NPE * CHUNK_T * KCH * P], _dt.float8e4, kind="ExternalInput"
    )
    idx_d = nc.dram_tensor("idx", [P, IDX_COLS], _dt.int16, kind="ExternalInput")
    out_d = nc.dram_tensor("out", [P, TPB], _dt.float32, kind="ExternalOutput")
    # second output: ||x||^2 per row-group (PE-chunk columns only)
    xx_d = nc.dram_tensor("xx", [P, TPB], _dt.float32, kind="ExternalOutput")

    # partition p <-> rows p*TPB + t for t in [0, TPB)
    x_v = x_d[:, :].rearrange("(p t) d -> p t d", p=P)
    # one-hot input, partition = class-in-chunk: [cls, c, t, k, row]
    oh_v = oh_d[:, :].rearrange(
        "p (c t k r) -> p c t k r", c=NPE, t=CHUNK_T, k=KCH
    )

    with TileContext(nc) as tc:
        with (
            tc.tile_pool(name="xin", bufs=4) as x_pool,
            tc.tile_pool(name="wq", bufs=5) as wq_pool,
            tc.tile_pool(name="scr", bufs=3) as scr_pool,
            tc.tile_pool(name="psum", bufs=8, space="PSUM") as psum_pool,
            tc.tile_pool(name="oh", bufs=3) as oh_pool,
            tc.tile_pool(name="small", bufs=1) as small,
        ):
            # fp8 table resident in SBUF: wtab[p, k, :] = w8[128k + p]
            wtab = small.tile([P, KCH, D], _dt.float8e4)
            nc.scalar.dma_start(
                out=wtab[:],
                in_=w8_d[:, :].rearrange("p (k d) -> p k d", k=KCH),
            )
            idx_t = small.tile([P, IDX_COLS], _dt.int16)
            nc.scalar.dma_start(out=idx_t[:], in_=idx_d[:, :])
            ssq = small.tile([P, TPB], _dt.float32)
            xxt = small.tile([P, TPB], _dt.float32)

            g0 = 0   # running row-group index
            pe_i = 0  # PE chunk ordinal
            for c, (kind, ct) in enumerate(CHUNKS):
                if kind == "pe":
                    # one-hot load issued before the (2 MB) x load so the
                    # PE feed is not FIFO-queued behind it
                    oh_t = oh_pool.tile([P, ct, KCH, P], _dt.float8e4)
                    nc.sync.dma_start(out=oh_t[:], in_=oh_v[:, pe_i, :ct])
                    pe_i += 1
                x_t = x_pool.tile([P, ct, D], _dt.float32, tag="x")
                nc.sync.dma_start(out=x_t[:], in_=x_v[:, g0 : g0 + ct, :])
                # algebra path everywhere: s = ||x||^2 - 2 x.w + ||w||^2.
                # ACT squares x as soon as it lands (needs neither psums
                # nor gather data, so it runs ahead of the fetch paths);
                # DVE does one fused x*w multiply+row-accumulate per
                # row-group (in place on the x slice); the host adds
                # ||w||^2 from targets.
                for t in range(ct):
                    g = g0 + t
                    sq_t = scr_pool.tile([P, D], _dt.bfloat16, tag="sq")
                    nc.scalar.activation(
                        out=sq_t[:],
                        in_=x_t[:, t, :],
                        func=mybir.ActivationFunctionType.Square,
                        accum_out=xxt[:, g : g + 1],
                    )
                if kind == "pe":
                    for t in range(ct):
                        g = g0 + t
                        ps = psum_pool.tile([P, D], _dt.float32, tag="ps")
                        for k2 in range(KCH // 2):
                            nc.tensor.matmul(
                                out=ps[:],
                                lhsT=oh_t[:, t, 2 * k2 : 2 * k2 + 2, :],
                                rhs=wtab[:, 2 * k2 : 2 * k2 + 2, :],
                                start=(k2 == 0),
                                stop=(k2 == KCH // 2 - 1),
                                perf_mode=mybir.MatmulPerfMode.DoubleRow,
                            )
                        nc.vector.scalar_tensor_tensor(
                            out=x_t[:, t, :],
                            in0=x_t[:, t, :],
                            scalar=0.0,
                            in1=ps[:],
                            op0=mybir.AluOpType.bypass,
                            op1=mybir.AluOpType.mult,
                            accum_out=ssq[:, g : g + 1],
                        )
                    g0 += ct
                    continue
                else:
                    icol0 = (g0 * P) // 16
                    icols = ct * P // 16
                    w_t = wq_pool.tile([P, ct, D], _dt.bfloat16, tag="wq")
                    nc.gpsimd.dma_gather(
                        out_ap=w_t[:],
                        in_ap=wq_d[:, :],
                        idxs_ap=idx_t[:, icol0 : icol0 + icols],
                        num_idxs=ct * P,
                        num_idxs_reg=ct * P,
                        elem_size=D,
                    )
                    for t in range(ct):
                        g = g0 + t
                        nc.vector.scalar_tensor_tensor(
                            out=x_t[:, t, :],
                            in0=x_t[:, t, :],
                            scalar=0.0,
                            in1=w_t[:, t, :],
                            op0=mybir.AluOpType.bypass,
                            op1=mybir.AluOpType.mult,
                            accum_out=ssq[:, g : g + 1],
                        )
                g0 += ct

            # ship per-row partials; host does the combine + sqrt + mean
            nc.sync.dma_start(out=out_d[:, :], in_=ssq[:])
            nc.sync.dma_start(out=xx_d[:, :], in_=xxt[:])
    nc.finalize()
    return nc


def _wrap_indices(targets_shard: np.ndarray) -> np.ndarray:
    """dma_gather index tensor [128, NSH//16] int16 (see docstring in
    the gather branch; only the Q7-path chunks' columns are used)."""
    tg = targets_shard.reshape(P, TPB)
    idx = np.zeros((P, IDX_COLS), np.int16)
    g0 = 0
    for kind, ct in CHUNKS:
        if kind == "g":
            arr = tg[:, g0 : g0 + ct].T.reshape(-1)  # slot-ordered
            wrap = arr.reshape(-1, 16).T             # [16, ct*8]
            c0 = (g0 * P) // 16
            idx[:, c0 : c0 + ct * P // 16] = np.tile(wrap, (8, 1))
        g0 += ct
    return idx


def _build_onehots(targets_shard: np.ndarray) -> np.ndarray:
    """One-hot selectors for the PE chunks: oh[cls, pe_i, t, k, p] = 1
    iff targets[row p*TPB + g0(pe_i) + t] == 128k + cls, flattened to
    [128, NPE*CHUNK_T*KCH*128] fp8."""
    tg = targets_shard.reshape(P, TPB)
    pe_spans = []
    g0 = 0
    for kind, ct in CHUNKS:
        if kind == "pe":
            pe_spans.append((g0, ct))
        g0 += ct
    oh = np.zeros((P, NPE, CHUNK_T, KCH, P), np.uint8)
    for pe_i, (s, ct) in enumerate(pe_spans):
        tcl = tg[:, s : s + ct]                  # [p, t]
        k = tcl // P
        cls = tcl % P
        pp, tt = np.meshgrid(np.arange(P), np.arange(ct), indexing="ij")
        oh[cls.ravel(), pe_i, tt.ravel(), k.ravel(), pp.ravel()] = 1
    one = np.uint8(ml_dtypes.float8_e4m3(1.0).view(np.uint8))
    return (oh * one).reshape(P, -1).view(ml_dtypes.float8_e4m3)


_NC_CACHE = None


def kernel(x, weight, targets):
    global _NC_CACHE
    x = np.ascontiguousarray(np.asarray(x, dtype=np.float32))
    weight = np.ascontiguousarray(np.asarray(weight, dtype=np.float32))
    targets = np.asarray(targets).astype(np.int64)
    assert x.shape == (N, D) and weight.shape == (C, D) and targets.shape == (N,)

    if _NC_CACHE is None:
        _NC_CACHE = _build_bass()
    nc = _NC_CACHE

    wq = np.ascontiguousarray(weight.astype(ml_dtypes.bfloat16))
    w8pad = np.zeros((CPAD, D), ml_dtypes.float8_e4m3)
    w8pad[:C] = weight.astype(ml_dtypes.float8_e4m3)
    # [p, k*D] SBUF layout: row p holds classes p, 128+p, ...
    w8 = np.ascontiguousarray(
        w8pad.reshape(KCH, P, D).transpose(1, 0, 2).reshape(P, KCH * D)
    )
    in_maps = []
    for k in range(NCORES):
        sl = slice(k * NSH, (k + 1) * NSH)
        tsh = targets[sl]
        in_maps.append(
            {
                "x": x[sl],
                "wq": wq,
                "w8": w8,
                "oh": _build_onehots(tsh),
                "idx": _wrap_indices(tsh),
            }
        )

    res = run_bass_kernel_spmd(nc, in_maps, core_ids=list(range(NCORES)))
    wsq = (weight.astype(np.float64) ** 2).sum(1)
    total = np.float64(0.0)
    for k, r in enumerate(res.results):
        tg = targets[k * NSH : (k + 1) * NSH].reshape(P, TPB)
        xw = r["out"].astype(np.float64)
        xx = r["xx"].astype(np.float64)
        s = xx - 2.0 * xw + wsq[tg]
        total += np.sqrt(s + EPS).sum()
    return np.float32(total / N)


if __name__ == "__main__":
    rng = np.random.default_rng(0)
    x = rng.standard_normal((N, D), dtype=np.float32)
    w = (rng.standard_normal((C, D)) / np.sqrt(D)).astype(np.float32)
    t = rng.integers(0, C, size=(N,)).astype(np.int64)
    got = kernel(x, w, t)
    wt = w[t]
    exp = np.sqrt(((x - wt) ** 2).sum(1) + EPS).mean()
    print("kernel:", got, "expected:", exp, "rel:", abs(got - exp) / abs(exp))
